# revision 2
# baseline (speedup 1.0000x reference)
"""Trainium2 Bass kernel for nn_MessagePassingBlock (GNN message passing), v5.

Math (reference):
    h     = x @ W_msg                       # (N, D)
    msg   = (h[source] + rel_bias[edge_type]) * edge_weights[:, None]
    delta = segment_sum(msg, target, N)     # (N, D)
    out   = relu(x @ W_self + delta + b)

Distribution: target-sharded across 8 cores (no collectives). Core c owns
nodes [c*12544, (c+1)*12544); every edge lives on its target's core.

v4 changes vs baseline:
  - target one-hot streamed from HBM (on-chip DVE build loses: SWDGE
    descriptor rings share SBUF ports with DVE, tripling DVE op cost
    during gathers).
  - rel-bias term via host-precomputed cnt_w[8, nodes] (summed edge
    weights per (relation, target)) folded into the epilogue matmul --
    deletes 784 cT matmuls + the ohe stream from the hot loop.
  - self-term x^T obtained by HWDGE DMA-transpose from the bf16 shard
    (no PE transposes, no DVE copies for it).
  - all matmul operands bf16; output written bf16 and upcast on host.
  - bias + ReLU fused into one scalar-engine activation (bias AP).
  - PSUM->SBUF copies moved to the scalar engine (ACT is idle, sits
    closer to PSUM).
  - one gather instruction per (superblock, subtable).
"""

import numpy as np
import ml_dtypes

NUM_NODES = 100000
D = 128
NUM_REL = 8
N_CORES = 8
NODES_PER_CORE = 12544          # 98 blocks of 128
NBLK = NODES_PER_CORE // 128    # 98
SB_BLOCKS = 14                  # blocks per superblock
N_SB = NBLK // SB_BLOCKS        # 7
N_SUBT = 4
SUBT_ROWS = 25000               # rows per gather subtable

_kernel_cache = {}


def _build_and_compile(c_bt_key, nchunks_sbt, chunk_plan):
    """Build + compile the SPMD Bass kernel for a given static chunk layout."""
    import concourse.bacc as bacc
    import concourse.tile as tile
    import concourse.mybir as mybir

    NC_TOT = sum(sum(row) for row in nchunks_sbt)

    nc = bacc.Bacc(
        "TRN2",
        target_bir_lowering=False,
        debug=False,
        num_devices=N_CORES,
        num_swdge_queues=4,
    )
    f32 = mybir.dt.float32
    bf16 = mybir.dt.bfloat16
    i16 = mybir.dt.int16

    xbf = nc.dram_tensor("xbf", [NUM_NODES, D], bf16, kind="ExternalInput")
    xsh = nc.dram_tensor("xsh", [NODES_PER_CORE, D], bf16, kind="ExternalInput")
    wmsg = nc.dram_tensor("wmsg", [D, D], bf16, kind="ExternalInput")
    wself = nc.dram_tensor("wself", [D, D], bf16, kind="ExternalInput")
    relb = nc.dram_tensor("relb", [NUM_REL, D], bf16, kind="ExternalInput")
    bcol = nc.dram_tensor("bcol", [D, 1], f32, kind="ExternalInput")
    n_idx_cols = sum(n * 128 // 16 for row in nchunks_sbt for n in row)
    gidx = nc.dram_tensor("gidx", [128, n_idx_cols], i16, kind="ExternalInput")
    ohw_meta = nc.dram_tensor("ohw_meta", [128, NC_TOT * 128], bf16, kind="ExternalInput")
    cntw = nc.dram_tensor("cntw", [NUM_REL, NODES_PER_CORE], bf16, kind="ExternalInput")
    out_d = nc.dram_tensor("out", [D, NODES_PER_CORE], bf16, kind="ExternalOutput")

    # static offsets
    idx_off = {}
    off = 0
    for sb in range(N_SB):
        for t in range(N_SUBT):
            idx_off[(sb, t)] = off
            off += nchunks_sbt[sb][t] * 128 // 16
    gmax = [max(nchunks_sbt[sb][t] for sb in range(N_SB)) for t in range(N_SUBT)]
    pos_of = {}
    _p = 0
    for _b in range(NBLK):
        pos_of[_b] = _p
        _p += len(chunk_plan[_b])
    assert _p == NC_TOT
    # max chunks in any 7-block half (for the ohw tile size)
    ghw_max = max(
        pos_of.get(g0 + 7, NC_TOT) - pos_of[g0] for g0 in range(0, NBLK, 7)
    )

    with tile.TileContext(nc) as tc:
        with tc.tile_pool(name="const", bufs=1) as cpool, tc.tile_pool(
            name="meta", bufs=1
        ) as mpool, tc.tile_pool(name="gath", bufs=3) as gpool, tc.tile_pool(
            name="oh", bufs=2
        ) as ohpool, tc.tile_pool(name="blk", bufs=3) as bpool, tc.tile_pool(
            name="xt", bufs=2
        ) as xtpool, tc.tile_pool(name="ps", bufs=2, space="PSUM") as pspool, \
            tc.tile_pool(name="pso", bufs=2, space="PSUM") as psopool:
            # ---- constants ----
            wmsg_b = cpool.tile([128, D], bf16)
            nc.sync.dma_start(out=wmsg_b[:], in_=wmsg.ap())
            wself_b = cpool.tile([128, D], bf16)
            nc.sync.dma_start(out=wself_b[:], in_=wself.ap())
            rb_b = cpool.tile([NUM_REL, D], bf16)
            nc.sync.dma_start(out=rb_b[:], in_=relb.ap())
            b_col = cpool.tile([D, 1], f32)
            nc.sync.dma_start(out=b_col[:], in_=bcol.ap())
            # ---- metadata ----
            gidx_t = mpool.tile([128, n_idx_cols], i16)
            sb_icol = []
            for sb in range(N_SB):
                lo = idx_off[(sb, 0)]
                hi = (idx_off[(sb + 1, 0)] if sb + 1 < N_SB else n_idx_cols)
                sb_icol.append((lo, hi))
                nc.sync.dma_start(
                    out=gidx_t[:, lo:hi], in_=gidx.ap()[:, lo:hi])
            cntw_t = mpool.tile([NUM_REL, NODES_PER_CORE], bf16)
            nc.scalar.dma_start(out=cntw_t[:], in_=cntw.ap())

            for sb in range(N_SB):
                # ---- one gather instruction per subtable ----
                gtiles = []
                for t in range(N_SUBT):
                    nck = nchunks_sbt[sb][t]
                    if nck == 0:
                        gtiles.append(None)
                        continue
                    gt = gpool.tile([128, gmax[t] * 128], bf16, tag=f"g{t}",
                                    name=f"g{t}")
                    base = t * SUBT_ROWS
                    rows = min(SUBT_ROWS, NUM_NODES - base)
                    io = idx_off[(sb, t)]
                    n = nck * 128
                    nc.gpsimd.dma_gather(
                        out_ap=gt[:, : n].rearrange("p (c r) -> p c r", r=128),
                        in_ap=xbf.ap()[base : base + rows, :],
                        idxs_ap=gidx_t[:, io : io + n // 16],
                        num_idxs=n,
                        num_idxs_reg=n,
                        elem_size=D,
                        single_packet=False,
                        queue_num=t,
                    )
                    gtiles.append(gt)

                for half in range(2):
                    g0 = sb * SB_BLOCKS + half * 7
                    p0 = pos_of[g0]
                    p1 = pos_of[g0 + 7] if g0 + 7 < NBLK else NC_TOT
                    nchv = p1 - p0
                    # ---- streamed target-onehot (HWDGE, scalar queue) ----
                    ghw = ohpool.tile([128, ghw_max * 128], bf16, tag="ghw",
                                      name="ghw")
                    nc.scalar.dma_start(
                        out=ghw[:, : nchv * 128],
                        in_=ohw_meta.ap()[:, p0 * 128 : p1 * 128],
                    )
                    # ---- x^T for the self term via DMA transpose ----
                    xT = xtpool.tile([128, 7 * 128], bf16, tag="xT", name="xT")
                    nc.scalar.dma_start(
                        out=xT[:],
                        in_=xsh.ap()[g0 * 128 : (g0 + 7) * 128, :],
                        transpose=True,
                    )
                    o7 = bpool.tile([128, 7 * 128], bf16, tag="o7", name="o7")
                    sT_p = {}
                    for bi in range(7):
                        blk = g0 + bi
                        plan = chunk_plan[blk]
                        assert plan, f"block {blk} has no chunks"
                        nchunk = len(plan)
                        bpos = pos_of[blk] - p0
                        sT = pspool.tile([128, 128], f32, tag="sT", name="sT")
                        sT_p[bi] = sT
                        for ci, (t, slot, gchunk) in enumerate(plan):
                            ohw = ghw[:, (bpos + ci) * 128 : (bpos + ci + 1) * 128]
                            xg = gtiles[t][:, slot * 128 : (slot + 1) * 128]
                            nc.tensor.matmul(
                                out=sT[:], lhsT=xg, rhs=ohw,
                                start=(ci == 0), stop=(ci == nchunk - 1),
                            )
                        if bi % 2 == 1 or bi == 6:
                            lo = bi - 1 if bi % 2 == 1 else bi
                            nb = bi - lo + 1
                            w = nb * 128
                            sT_sb = bpool.tile([128, 256], bf16, tag="sTsb",
                                               name="sTsb")
                            for k2 in range(nb):
                                b2 = lo + k2
                                nc.scalar.activation(
                                    out=sT_sb[:, k2 * 128 : (k2 + 1) * 128],
                                    in_=sT_p[b2][:],
                                    func=mybir.ActivationFunctionType.Copy,
                                )
                            accT = psopool.tile([128, 256], f32, tag="accT",
                                                name="accT")
                            nc.tensor.matmul(
                                out=accT[:, :w], lhsT=wmsg_b[:], rhs=sT_sb[:, :w],
                                start=True, stop=False,
                            )
                            nc.tensor.matmul(
                                out=accT[:, :w], lhsT=rb_b[:],
                                rhs=cntw_t[:, (g0 + lo) * 128 : (g0 + lo) * 128 + w],
                                start=False, stop=False,
                            )
                            nc.tensor.matmul(
                                out=accT[:, :w], lhsT=wself_b[:],
                                rhs=xT[:, lo * 128 : lo * 128 + w],
                                start=False, stop=True,
                            )
                            nc.scalar.activation(
                                out=o7[:, lo * 128 : lo * 128 + w],
                                in_=accT[:, :w],
                                func=mybir.ActivationFunctionType.Relu,
                                bias=b_col[:, 0:1],
                            )
                    nc.sync.dma_start(
                        out=out_d.ap()[:, g0 * 128 : (g0 + 7) * 128],
                        in_=o7[:],
                    )

    nc.compile()
    return nc


def _prep(inputs):
    """Host-side sharding/layout. Returns (in_maps, static_key, layout)."""
    x = np.ascontiguousarray(np.asarray(inputs["x"], dtype=np.float32))
    source = np.asarray(inputs["source"]).astype(np.int64)
    target = np.asarray(inputs["target"]).astype(np.int64)
    edge_type = np.asarray(inputs["edge_type"]).astype(np.int64)
    ew = np.asarray(inputs["edge_weights"], dtype=np.float32)
    w_msg = np.asarray(inputs["W_msg"], dtype=np.float32)
    rel_bias = np.asarray(inputs["rel_bias"], dtype=np.float32)
    w_self = np.asarray(inputs["W_self"], dtype=np.float32)
    b = np.asarray(inputs["b"], dtype=np.float32).reshape(D, 1)

    n = x.shape[0]
    assert n == NUM_NODES

    xbf = x.astype(ml_dtypes.bfloat16)

    core = target // NODES_PER_CORE
    tgt_local = target - core * NODES_PER_CORE
    blk = tgt_local >> 7
    tgt_in_blk = (tgt_local & 127).astype(np.int64)
    subt = source // SUBT_ROWS
    src_local = (source - subt * SUBT_ROWS).astype(np.int64)

    key = ((core * NBLK + blk) * N_SUBT + subt).astype(np.int64)
    order = np.argsort(key, kind="stable")
    key_s = key[order]
    uniq, starts = np.unique(key_s, return_index=True)
    counts = np.diff(np.append(starts, key_s.shape[0]))

    cnt = np.zeros((N_CORES, NBLK, N_SUBT), dtype=np.int64)
    ci_ = uniq // (NBLK * N_SUBT)
    bi_ = (uniq // N_SUBT) % NBLK
    ti_ = uniq % N_SUBT
    cnt[ci_, bi_, ti_] = counts

    # static chunk capacity per (blk, subtable): max over cores
    c_bt = np.ceil(cnt.max(axis=0) / 128).astype(np.int64)  # (NBLK, N_SUBT)
    empty = c_bt.sum(axis=1) == 0
    c_bt[empty, 0] = 1

    nchunks_sbt = [
        [int(c_bt[sb * SB_BLOCKS : (sb + 1) * SB_BLOCKS, t].sum())
         for t in range(N_SUBT)]
        for sb in range(N_SB)
    ]
    NC_TOT = int(c_bt.sum())

    # slot/chunk bookkeeping (same indexing as baseline)
    gchunk_of = np.zeros((NBLK, N_SUBT), dtype=np.int64)
    slot_of = np.zeros((NBLK, N_SUBT), dtype=np.int64)
    g = 0
    for sb in range(N_SB):
        for t in range(N_SUBT):
            s = 0
            for bi2 in range(SB_BLOCKS):
                bb = sb * SB_BLOCKS + bi2
                gchunk_of[bb, t] = g
                slot_of[bb, t] = s
                g += int(c_bt[bb, t])
                s += int(c_bt[bb, t])
    assert g == NC_TOT

    chunk_plan = []
    for bb in range(NBLK):
        plan = []
        for t in range(N_SUBT):
            for c in range(int(c_bt[bb, t])):
                plan.append((t, int(slot_of[bb, t] + c), int(gchunk_of[bb, t] + c)))
        chunk_plan.append(plan)

    pos_of_blk = np.zeros(NBLK, dtype=np.int64)
    p = 0
    for bb in range(NBLK):
        pos_of_blk[bb] = p
        p += len(chunk_plan[bb])
    # block-major position of each (b, t, c) chunk
    # prefix of chunks within the block, per subtable
    off_bt = np.zeros((NBLK, N_SUBT), dtype=np.int64)
    for bb in range(NBLK):
        acc = 0
        for t in range(N_SUBT):
            off_bt[bb, t] = acc
            acc += int(c_bt[bb, t])

    n_idx_cols = sum(nc_ * 128 // 16 for row in nchunks_sbt for nc_ in row)

    # ---- per-edge slot assignment (vectorized per core) ----
    in_maps = []
    edge_core = core[order]
    edge_blk = blk[order]
    edge_subt = subt[order]
    edge_src_local = src_local[order]
    edge_tgt_in_blk = tgt_in_blk[order]
    edge_ew = ew[order]
    edge_rel = edge_type[order]
    # rank within (core, blk, subt) group
    grp_start = np.repeat(starts, counts)
    rank = np.arange(len(order)) - grp_start

    wmsg_bf = np.ascontiguousarray(w_msg.astype(ml_dtypes.bfloat16))
    wself_bf = np.ascontiguousarray(w_self.astype(ml_dtypes.bfloat16))
    relb_bf = np.ascontiguousarray(rel_bias.astype(ml_dtypes.bfloat16))

    for c in range(N_CORES):
        m = edge_core == c
        e_blk = edge_blk[m]
        e_subt = edge_subt[m]
        e_srcl = edge_src_local[m]
        e_tib = edge_tgt_in_blk[m]
        e_w = edge_ew[m]
        e_rel = edge_rel[m]
        e_rank = rank[m]

        # chunk within (b,t) and partition
        e_chunk = e_rank >> 7
        e_part = e_rank & 127
        # gather slot position ((sb,t)-major slot space)
        e_slot = (slot_of[e_blk, e_subt] + e_chunk) * 128 + e_part
        # block-major chunk position
        e_pos = pos_of_blk[e_blk] + off_bt[e_blk, e_subt] + e_chunk

        # gidx: wrapped by (sb, t) instruction
        gidx_arr = np.zeros((128, n_idx_cols), dtype=np.int16)
        icol = 0
        for sb in range(N_SB):
            for t in range(N_SUBT):
                nck = nchunks_sbt[sb][t]
                if nck == 0:
                    continue
                nslots = nck * 128
                sel = (e_subt == t) & (e_blk >= sb * SB_BLOCKS) & (
                    e_blk < (sb + 1) * SB_BLOCKS)
                idxs = np.zeros(nslots, dtype=np.int16)
                loc = e_slot[sel] - int(slot_of[sb * SB_BLOCKS, t]) * 128
                idxs[loc] = e_srcl[sel].astype(np.int16)
                wrapped = idxs.reshape(nslots // 16, 16).T
                gidx_arr[:, icol : icol + nslots // 16] = np.tile(wrapped, (8, 1))
                icol += nslots // 16
        assert icol == n_idx_cols

        ohw_arr = np.zeros((128, NC_TOT * 128), dtype=ml_dtypes.bfloat16)
        ohw_arr[e_part, e_pos * 128 + e_tib] = e_w.astype(ml_dtypes.bfloat16)
        cntw_arr = np.zeros((NUM_REL, NODES_PER_CORE), dtype=np.float32)
        e_tloc = e_blk * 128 + e_tib
        np.add.at(cntw_arr, (e_rel, e_tloc), e_w)
        cntw_arr = cntw_arr.astype(ml_dtypes.bfloat16)

        xs = np.zeros((NODES_PER_CORE, D), dtype=ml_dtypes.bfloat16)
        lo = c * NODES_PER_CORE
        hi = min(lo + NODES_PER_CORE, NUM_NODES)
        xs[: hi - lo] = xbf[lo:hi]

        in_maps.append(
            {
                "xbf": xbf,
                "xsh": np.ascontiguousarray(xs),
                "wmsg": wmsg_bf,
                "wself": wself_bf,
                "relb": relb_bf,
                "bcol": np.ascontiguousarray(b),
                "gidx": gidx_arr,
                "ohw_meta": ohw_arr,
                "cntw": np.ascontiguousarray(cntw_arr),
            }
        )

    static_key = tuple(c_bt.flatten().tolist())
    return in_maps, static_key, (nchunks_sbt, chunk_plan)


def kernel(**inputs) -> np.ndarray:
    from concourse import bass_utils

    in_maps, static_key, (nchunks_sbt, chunk_plan) = _prep(inputs)

    nc = _kernel_cache.get(static_key)
    if nc is None:
        nc = _build_and_compile(static_key, nchunks_sbt, chunk_plan)
        _kernel_cache[static_key] = nc

    res = bass_utils.run_bass_kernel_spmd(
        nc, in_maps, core_ids=list(range(N_CORES))
    )
    parts = [
        np.asarray(res.results[c]["out"]).astype(np.float32).T
        for c in range(N_CORES)
    ]
    full = np.concatenate(parts, axis=0)[:NUM_NODES]
    return np.ascontiguousarray(full, dtype=np.float32)
